# revision 5
# baseline (speedup 1.0000x reference)
"""Trainium2 Bass kernel for nn_BPPSModel (type-routed atom MLP + segment pooling).

Strategy:
- Atoms sharded contiguously across 8 cores (50000 each), each core split into
  2 blocks of 25000 so gather indices fit int16.
- Host folds the LayerNorm mean-subtraction into the weights (W - rowwise mean
  over output dim) and exploits LN scale-invariance (g=1, b=0 path): layer-1's
  inverse-sigma cancels inside layer-2's LayerNorm; layer-2's inverse-sigma is
  applied on the host from a device-computed sum-of-squares.
- Features are split into bf16 hi + lo planes on the host (same total bytes as
  fp32). The device gathers atoms type-sorted AND transposed in one DMA
  (dma_gather transpose=True) - the MoE dispatch - then runs weight-stationary
  matmuls with the 3-term bf16 scheme (xh*Wh + xh*Wl + xl*Wh, rel err ~4e-6).
- Per-atom energy e' = wout . relu(z2c) (fp32 matmul) and v = sum(z2c^2)
  (bf16 matmul vs ones) are computed with M=1 PE reduce-matmuls; the host
  applies e = e' * rsqrt(v/256 + eps) and pools with bincount per structure,
  summing partials across the 8 cores.
"""

import numpy as np
import ml_dtypes

N_ATOMS = 400000
N_FEAT = 512
H1 = 256
H2 = 256
N_TYPES = 4
NUM_STRUCTS = 4096
LN_EPS = 1e-5
N_CORES = 8
ATOMS_PER_CORE = N_ATOMS // N_CORES
BLOCKS_PER_CORE = 2
BLOCK = ATOMS_PER_CORE // BLOCKS_PER_CORE  # 25000
TILE_A = 512  # atoms per tile (free dim)

_cache = {}


def _numpy_reference(features, W1, W2, Wout, g1, b1, g2, b2, comp_w, numbers, batch):
    x = features.astype(np.float32)
    t = numbers.astype(np.int64)

    def linmap(h, W):
        out = np.zeros((h.shape[0], W.shape[2]), dtype=np.float32)
        for ty in range(W.shape[0]):
            m = t == ty
            out[m] = h[m] @ W[ty]
        return out

    def ln(h, g, b):
        mu = h.mean(axis=-1, keepdims=True)
        var = h.var(axis=-1, keepdims=True)
        return (h - mu) / np.sqrt(var + LN_EPS) * g + b

    h = np.maximum(ln(linmap(x, W1), g1, b1), 0.0)
    h = np.maximum(ln(linmap(h, W2), g2, b2), 0.0)
    atom_e = linmap(h, Wout)[:, 0]
    energies = np.bincount(batch.astype(np.int64), weights=atom_e, minlength=NUM_STRUCTS)
    onehot_w = comp_w[0].astype(np.float64)[t]
    comp = np.bincount(batch.astype(np.int64), weights=onehot_w, minlength=NUM_STRUCTS)
    return (energies + comp).reshape(NUM_STRUCTS, 1).astype(np.float32)


def _build_schedule(numbers):
    """Per-core, per-block type sort with runs padded to TILE_A multiples.

    Returns (tile_bt schedule common to all cores, per-core idx/valid/perm)."""
    numbers = numbers.astype(np.int64)
    counts = np.zeros((N_CORES, BLOCKS_PER_CORE, N_TYPES), dtype=np.int64)
    sorts = []
    for c in range(N_CORES):
        row = []
        for b in range(BLOCKS_PER_CORE):
            lo = c * ATOMS_PER_CORE + b * BLOCK
            nb = numbers[lo : lo + BLOCK]
            order = np.argsort(nb, kind="stable")
            row.append(order)
            counts[c, b] = np.bincount(nb, minlength=N_TYPES)
        sorts.append(row)
    ktiles = np.zeros((BLOCKS_PER_CORE, N_TYPES), dtype=np.int64)
    for b in range(BLOCKS_PER_CORE):
        for t in range(N_TYPES):
            ktiles[b, t] = int(np.ceil(counts[:, b, t].max() / TILE_A))
    n_tiles = int(ktiles.sum())
    assert n_tiles <= 128, n_tiles

    per_core = []
    for c in range(N_CORES):
        idx_rel = np.zeros((n_tiles, TILE_A), dtype=np.int16)
        valid = np.zeros((n_tiles, TILE_A), dtype=bool)
        perm_global = np.zeros((n_tiles, TILE_A), dtype=np.int64)
        j = 0
        for b in range(BLOCKS_PER_CORE):
            order = sorts[c][b]
            base = c * ATOMS_PER_CORE + b * BLOCK
            off = 0
            for t in range(N_TYPES):
                cnt = int(counts[c, b, t])
                run = order[off : off + cnt]
                off += cnt
                for k in range(int(ktiles[b, t])):
                    seg = run[k * TILE_A : (k + 1) * TILE_A]
                    n = len(seg)
                    idx_rel[j, :n] = seg.astype(np.int16)
                    valid[j, :n] = True
                    perm_global[j, :n] = base + seg
                    j += 1
        per_core.append(dict(idx=idx_rel, valid=valid, perm=perm_global))
    tile_bt = []
    for b in range(BLOCKS_PER_CORE):
        for t in range(N_TYPES):
            for _ in range(int(ktiles[b, t])):
                tile_bt.append((b, t))
    return tile_bt, per_core


def _wrap_idx(idx_rel):
    """[T, 512] int16 -> [128, T*32]: index i -> partition i%16, slot i//16,
    replicated across the 8 gpsimd core groups."""
    T = idx_rel.shape[0]
    out = np.zeros((128, T, 32), dtype=np.int16)
    w = idx_rel.reshape(T, 32, 16)  # [T, slot, lane]
    for rep in range(8):
        out[16 * rep : 16 * rep + 16] = np.transpose(w, (2, 0, 1))
    return out.reshape(128, T * 32)


def _build_module(tile_bt):
    import concourse.tile as tile
    from concourse import bacc, mybir
    from concourse import library_config

    F32 = mybir.dt.float32
    BF16 = mybir.dt.bfloat16
    I16 = mybir.dt.int16
    AF = mybir.ActivationFunctionType

    T = len(tile_bt)
    nc = bacc.Bacc(
        "TRN2", target_bir_lowering=False, debug=False, num_devices=N_CORES,
        enable_asserts=False,
    )
    xh = [
        nc.dram_tensor(f"xh{b}", [BLOCK, N_FEAT], BF16, kind="ExternalInput")
        for b in range(BLOCKS_PER_CORE)
    ]
    xl = [
        nc.dram_tensor(f"xl{b}", [BLOCK, N_FEAT], BF16, kind="ExternalInput")
        for b in range(BLOCKS_PER_CORE)
    ]
    idx_in = nc.dram_tensor("idx", [128, T * 32], I16, kind="ExternalInput")
    w1h_in = nc.dram_tensor("w1h", [N_TYPES, N_FEAT, H1], BF16, kind="ExternalInput")
    w1l_in = nc.dram_tensor("w1l", [N_TYPES, N_FEAT, H1], BF16, kind="ExternalInput")
    w2h_in = nc.dram_tensor("w2h", [N_TYPES, H1, H2], BF16, kind="ExternalInput")
    w2l_in = nc.dram_tensor("w2l", [N_TYPES, H1, H2], BF16, kind="ExternalInput")
    wo_in = nc.dram_tensor("wo", [N_TYPES, H2], F32, kind="ExternalInput")
    ones_in = nc.dram_tensor("ones_bf", [128, 1], BF16, kind="ExternalInput")
    e_out = nc.dram_tensor("e_out", [128, 512], F32, kind="ExternalOutput")
    v_out = nc.dram_tensor("v_out", [128, 512], F32, kind="ExternalOutput")

    KF = N_FEAT // 128  # 4
    K2 = H1 // 128  # 2
    O1 = H1 // 128  # 2
    O2 = H2 // 128  # 2

    with tile.TileContext(nc) as tc:
        with (
            tc.tile_pool(name="const", bufs=1) as cp,
            tc.tile_pool(name="work", bufs=2) as wp,
            tc.tile_pool(name="psz", bufs=1, space="PSUM") as psz,
            tc.tile_pool(name="psr", bufs=2, space="PSUM") as psr,
        ):
            nc.gpsimd.load_library(library_config.mlp)

            w1h = cp.tile([128, N_TYPES, KF, O1, 128], BF16)
            nc.sync.dma_start(
                w1h[:], w1h_in.ap().rearrange("t (k p) (o q) -> p t k o q", p=128, q=128)
            )
            w1l = cp.tile([128, N_TYPES, KF, O1, 128], BF16)
            nc.sync.dma_start(
                w1l[:], w1l_in.ap().rearrange("t (k p) (o q) -> p t k o q", p=128, q=128)
            )
            w2h = cp.tile([128, N_TYPES, K2, O2, 128], BF16)
            nc.sync.dma_start(
                w2h[:], w2h_in.ap().rearrange("t (k p) (o q) -> p t k o q", p=128, q=128)
            )
            w2l = cp.tile([128, N_TYPES, K2, O2, 128], BF16)
            nc.sync.dma_start(
                w2l[:], w2l_in.ap().rearrange("t (k p) (o q) -> p t k o q", p=128, q=128)
            )
            wof = cp.tile([128, N_TYPES, K2, 1], F32)
            nc.sync.dma_start(
                wof[:], wo_in.ap().rearrange("t (k p) -> p t k", p=128).rearrange("p t k -> p t k ()")
            )
            ones_bf = cp.tile([128, 1], BF16)
            nc.sync.dma_start(ones_bf[:], ones_in.ap())
            idxs = cp.tile([128, T, 32], I16)
            nc.sync.dma_start(
                idxs[:], idx_in.ap().rearrange("p (t w) -> p t w", w=32)
            )

            srcs_h = [x.ap() for x in xh]
            srcs_l = [x.ap() for x in xl]

            for j, (b, t) in enumerate(tile_bt):
                gh = wp.tile([128, KF, TILE_A], BF16, tag="gh")
                nc.gpsimd.dma_gather(
                    out_ap=gh[:], in_ap=srcs_h[b], idxs_ap=idxs[:, j],
                    num_idxs=TILE_A, num_idxs_reg=TILE_A, elem_size=N_FEAT,
                    transpose=True,
                )
                gl = wp.tile([128, KF, TILE_A], BF16, tag="gl")
                nc.gpsimd.dma_gather(
                    out_ap=gl[:], in_ap=srcs_l[b], idxs_ap=idxs[:, j],
                    num_idxs=TILE_A, num_idxs_reg=TILE_A, elem_size=N_FEAT,
                    transpose=True,
                )

                # layer 1: z1 = xh*W1h + xh*W1l + xl*W1h   (3-term bf16)
                z1 = psz.tile([128, O1, TILE_A], F32, tag="z1")
                for o in range(O1):
                    n_mm = 3 * KF
                    i = 0
                    for k in range(KF):
                        nc.tensor.matmul(
                            z1[:, o], w1h[:, t, k, o], gh[:, k],
                            start=(i == 0), stop=(i == n_mm - 1),
                        )
                        i += 1
                    for k in range(KF):
                        nc.tensor.matmul(
                            z1[:, o], w1l[:, t, k, o], gh[:, k],
                            start=False, stop=(i == n_mm - 1),
                        )
                        i += 1
                    for k in range(KF):
                        nc.tensor.matmul(
                            z1[:, o], w1h[:, t, k, o], gl[:, k],
                            start=False, stop=(i == n_mm - 1),
                        )
                        i += 1

                # r1 = relu(z1): f32 (for lo extraction) + bf16 hi/lo pair
                r1f = wp.tile([128, O1, TILE_A], F32, tag="r1f")
                r1h = wp.tile([128, O1, TILE_A], BF16, tag="r1h")
                r1l = wp.tile([128, O1, TILE_A], BF16, tag="r1l")
                for o in range(O1):
                    nc.scalar.activation(r1f[:, o], z1[:, o], AF.Relu)
                    nc.vector.tensor_copy(r1h[:, o], r1f[:, o])
                    nc.vector.tensor_sub(r1l[:, o], r1f[:, o], r1h[:, o])

                # layer 2: z2 = r1h*W2h + r1h*W2l + r1l*W2h
                z2 = psz.tile([128, O2, TILE_A], F32, tag="z2")
                for o in range(O2):
                    n_mm = 3 * K2
                    i = 0
                    for k in range(K2):
                        nc.tensor.matmul(
                            z2[:, o], w2h[:, t, k, o], r1h[:, k],
                            start=(i == 0), stop=(i == n_mm - 1),
                        )
                        i += 1
                    for k in range(K2):
                        nc.tensor.matmul(
                            z2[:, o], w2l[:, t, k, o], r1h[:, k],
                            start=False, stop=(i == n_mm - 1),
                        )
                        i += 1
                    for k in range(K2):
                        nc.tensor.matmul(
                            z2[:, o], w2h[:, t, k, o], r1l[:, k],
                            start=False, stop=(i == n_mm - 1),
                        )
                        i += 1

                r2f = wp.tile([128, O2, TILE_A], F32, tag="r2f")
                sq = wp.tile([128, O2, TILE_A], BF16, tag="sq")
                for o in range(O2):
                    nc.scalar.activation(r2f[:, o], z2[:, o], AF.Relu)
                    nc.scalar.activation(sq[:, o], z2[:, o], AF.Square)

                # e' = wout . r2 (fp32), v = ones . z2^2 (bf16)
                e_ps = psr.tile([1, TILE_A], F32, tag="e")
                for k in range(K2):
                    nc.tensor.matmul(
                        e_ps[:], wof[:, t, k], r2f[:, k],
                        start=(k == 0), stop=(k == K2 - 1),
                    )
                v_ps = psr.tile([1, TILE_A], F32, tag="v")
                for k in range(O2):
                    nc.tensor.matmul(
                        v_ps[:], ones_bf[:], sq[:, k],
                        start=(k == 0), stop=(k == O2 - 1),
                    )
                tmp_e = wp.tile([1, TILE_A], F32, tag="tmp_e")
                nc.scalar.copy(tmp_e[:], e_ps[:])
                nc.sync.dma_start(e_out.ap()[j : j + 1, :], tmp_e[:])
                tmp_v = wp.tile([1, TILE_A], F32, tag="tmp_v")
                nc.vector.tensor_copy(tmp_v[:], v_ps[:])
                nc.sync.dma_start(v_out.ap()[j : j + 1, :], tmp_v[:])


    nc.compile()
    return nc


def _device_run(features, W1, W2, Wout, comp_w, numbers, batch):
    from concourse import bass_utils

    W1c = W1 - W1.mean(axis=2, keepdims=True)
    W2c = W2 - W2.mean(axis=2, keepdims=True)
    wo = np.ascontiguousarray(Wout[:, :, 0])

    def split(w):
        h = w.astype(ml_dtypes.bfloat16)
        l = (w - h.astype(np.float32)).astype(ml_dtypes.bfloat16)
        return h, l

    w1h, w1l = split(W1c)
    w2h, w2l = split(W2c)
    xh = features.astype(ml_dtypes.bfloat16)
    xl = (features - xh.astype(np.float32)).astype(ml_dtypes.bfloat16)

    tile_bt, per_core = _build_schedule(numbers)
    key = tuple(tile_bt)
    if key not in _cache:
        _cache[key] = _build_module(tile_bt)
    nc = _cache[key]

    ones_bf = np.ones((128, 1), dtype=ml_dtypes.bfloat16)
    in_maps = []
    for c in range(N_CORES):
        lo = c * ATOMS_PER_CORE
        im = dict(
            idx=_wrap_idx(per_core[c]["idx"]),
            w1h=w1h, w1l=w1l, w2h=w2h, w2l=w2l, wo=wo, ones_bf=ones_bf,
        )
        for b in range(BLOCKS_PER_CORE):
            s = lo + b * BLOCK
            im[f"xh{b}"] = np.ascontiguousarray(xh[s : s + BLOCK])
            im[f"xl{b}"] = np.ascontiguousarray(xl[s : s + BLOCK])
        in_maps.append(im)

    res = bass_utils.run_bass_kernel_spmd(nc, in_maps, core_ids=list(range(N_CORES)))

    T = len(tile_bt)
    energies = np.zeros(NUM_STRUCTS, dtype=np.float64)
    for c in range(N_CORES):
        e = res.results[c]["e_out"][:T].astype(np.float64)
        v = res.results[c]["v_out"][:T].astype(np.float64)
        inv = 1.0 / np.sqrt(np.maximum(v, 0.0) / H2 + LN_EPS)
        ea = e * inv
        valid = per_core[c]["valid"]
        perm = per_core[c]["perm"]
        energies += np.bincount(
            batch[perm[valid]], weights=ea[valid], minlength=NUM_STRUCTS
        )
    comp = np.bincount(
        batch, weights=comp_w[0].astype(np.float64)[numbers], minlength=NUM_STRUCTS
    )
    return (energies + comp).reshape(NUM_STRUCTS, 1).astype(np.float32)


def kernel(**inputs):
    features = np.asarray(inputs["features"], dtype=np.float32)
    W1 = np.asarray(inputs["W1"], dtype=np.float32)
    W2 = np.asarray(inputs["W2"], dtype=np.float32)
    Wout = np.asarray(inputs["Wout"], dtype=np.float32)
    g1 = np.asarray(inputs["g1"], dtype=np.float32)
    b1 = np.asarray(inputs["b1"], dtype=np.float32)
    g2 = np.asarray(inputs["g2"], dtype=np.float32)
    b2 = np.asarray(inputs["b2"], dtype=np.float32)
    comp_w = np.asarray(inputs["comp_w"], dtype=np.float32)
    numbers = np.asarray(inputs["numbers"]).astype(np.int64)
    batch = np.asarray(inputs["batch"]).astype(np.int64)

    fast_ok = (
        features.shape == (N_ATOMS, N_FEAT)
        and W1.shape == (N_TYPES, N_FEAT, H1)
        and np.all(g1 == 1.0) and np.all(b1 == 0.0)
        and np.all(g2 == 1.0) and np.all(b2 == 0.0)
    )
    if fast_ok:
        try:
            return _device_run(features, W1, W2, Wout, comp_w, numbers, batch)
        except Exception:
            import traceback

            traceback.print_exc()
    return _numpy_reference(
        features, W1, W2, Wout, g1, b1, g2, b2, comp_w, numbers, batch
    )
